# revision 12
# baseline (speedup 1.0000x reference)
"""Trainium2 Bass kernel for nn_DCTFeatureModel.

Math: the reference pipeline (3D DCT-II over [time-in-bin, H, W], mean over
DCT bins, full-receptive-field Conv3d, bias, LeakyReLU) is linear up to the
LeakyReLU, so everything folds into a single small matmul:

    feat[b,s,o] = LeakyReLU( sum_{c,k} x[b,s,c,k] * Weff[s,o,k] + bias[s,o] )
    Weff[s,o,k=(t,i,j)] = (1/8) * sum_{f,p,q} Ct[f,t] Cs[p,i] Cs[q,j] W[s,o,f,p,q]

Weff is tiny (2*64*2048 floats) and computed on host. The device kernel is
memory-bound: stream x (134 MB full / 16.8 MB per core) once from HBM.

Device dataflow (per core): host lays the x shard out as contiguous blocks
[kin=128, cc=2, b=128, c2=4] per (s, k-chunk) where the DCT-bin index is
c = cc*4 + c2, grouped into 2MB quad-tiles. The whole shard is resident in
SBUF (128 KB of the 208 KB per partition), all DMAs are issued up-front on
the sync HWDGE queue with zero buffer-reuse waits, so it streams at HBM
line rate (~420 GB/s) start to finish; the last quad is split into 4
chunk-sized DMAs so the PE tail after the final byte is minimal. The PE
consumes tiles directly with w-stationary float32r matmuls (moving dim 512
-> 1 cycle/row): per k-chunk both cc halves accumulate into one [64, 512]
PSUM bank per subwindow whose columns are (b, c2) -- so the 8 DCT bins
arrive pairwise-folded to 4 contiguous c2 lanes, and bias/4 is planted in
every column by a rank-1 matmul at accumulation start (start=True, long
before the tail). The epilogue per subwindow is 3 ops: one DVE
tensor_reduce over the contiguous c2 lanes ([64,512] -> [64,128]) and an
exact LeakyReLU = max(v, 0.02*v) via DVE mul + max.

Sharding: pure data-parallel over batch, 1024/8 = 128 rows per core.
"""

from contextlib import ExitStack

import numpy as np

import concourse.bacc as bacc
import concourse.tile as tile
from concourse import mybir
from concourse.bass_utils import run_bass_kernel_spmd

# Problem shapes (hardcoded per contract)
B = 1024
NCORES = 8
BS = B // NCORES          # 128 batch rows per core
NSW = 2                   # subwindows
NBINS = 8                 # DCT bins (mean-reduced)
NDCT = 32                 # time points per bin
HW = 8
NF = 64                   # conv output filters per subwindow
K = NDCT * HW * HW        # 2048 contraction elements per (s, c)
P = 128                   # partitions
NCHUNK = K // P           # 16 k-chunks of 128
QUAD = 4                  # chunks per big DMA tile
NQ = NCHUNK // QUAD       # 4 quads per subwindow
NTILE = NSW * NQ          # 8 quad positions; the last is split chunk-wise
TILE_W = QUAD * NBINS * BS  # 4096 cols per quad tile: [ch4, cc, b, c2]
CW = NBINS * BS           # 1024 cols per chunk: [cc, b, c2]
HALF = CW // 2            # 512-column matmul (one PSUM bank)
C2 = 4                    # folded c lanes kept in PSUM columns
OUT_F = NSW * NF          # 128 output features
SLOPE = 0.02

F32 = mybir.dt.float32
F32R = mybir.dt.float32r

_cached = None
last_results = None


def _dct2(N):
    n = np.arange(N, dtype=np.float64)
    k = np.arange(N, dtype=np.float64)
    return 2.0 * np.cos(np.pi * (2.0 * n[None, :] + 1.0) * k[:, None] / (2.0 * N))


def _kernel_body(tc, x, xtail, w, bias4, ones, out):
    """x: [NTILE-1, P, TILE_W] (s-major quads, cols [ch4, cc, b, c2])
    xtail: [QUAD, P, CW] (the last quad, chunk-granular)
    w: [P, NSW*NCHUNK*NF] (f32r); bias4: [1, OUT_F] (f32r, bias/C2);
    ones: [1, HALF] (f32r); out: [NF, NSW*BS] (cols [s, b])."""
    nc = tc.nc
    with ExitStack() as ctx:
        pool = ctx.enter_context(tc.tile_pool(name="sb", bufs=1))
        ppool = ctx.enter_context(tc.tile_pool(name="pp", bufs=1, space="PSUM"))

        # consts on the scalar HWDGE ring; the x stream owns the sync ring
        w_sb = pool.tile([P, NSW * NCHUNK * NF], F32R)
        nc.scalar.dma_start(out=w_sb, in_=w)
        bias_sb = pool.tile([1, OUT_F], F32R)
        nc.scalar.dma_start(out=bias_sb, in_=bias4)
        ones_sb = pool.tile([1, HALF], F32R)
        nc.scalar.dma_start(out=ones_sb, in_=ones)

        out_sb = pool.tile([NF, NSW * BS], F32)

        # the full x shard: 7 x 2MB quads + 4 x 512KB chunks, all DMAs issued
        # up-front on one queue, every buffer used exactly once
        xt = []
        for i in range(NTILE - 1):
            t = pool.tile([P, TILE_W], F32R, tag=f"x{i}", name=f"x_{i}")
            nc.sync.dma_start(out=t, in_=x[i])
            xt.append(t)
        xtl = []
        for j in range(QUAD):
            t = pool.tile([P, CW], F32R, tag=f"xt{j}", name=f"xt_{j}")
            nc.sync.dma_start(out=t, in_=xtail[j])
            xtl.append(t)

        def chunk_half(ch, cc):
            """rhs AP for global chunk index ch, cc half."""
            ti = ch // QUAD
            j = ch % QUAD
            base = xt[ti] if ti < NTILE - 1 else xtl[j]
            off = (j * CW if ti < NTILE - 1 else 0) + cc * HALF
            return base[:, off:off + HALF]

        for s in range(NSW):
            # one PSUM bank per subwindow, columns (b, c2); bias/4 is planted
            # in every column up-front (start=True), all chunks accumulate
            ps = ppool.tile([NF, HALF], F32, tag=f"ps{s}", name=f"ps{s}")
            nc.tensor.matmul(ps, lhsT=bias_sb[:, s * NF:(s + 1) * NF],
                             rhs=ones_sb, start=True, stop=False)
            for ch in range(NCHUNK):
                wv = w_sb[:, (s * NCHUNK + ch) * NF:(s * NCHUNK + ch + 1) * NF]
                gch = s * NCHUNK + ch
                nc.tensor.matmul(ps, lhsT=wv, rhs=chunk_half(gch, 0),
                                 start=False, stop=False)
                nc.tensor.matmul(ps, lhsT=wv, rhs=chunk_half(gch, 1),
                                 start=False, stop=(ch == NCHUNK - 1))

            # epilogue: fold the 4 contiguous c2 lanes in one DVE reduce,
            # then exact LeakyReLU = max(v, 0.02v)
            r = pool.tile([NF, BS], F32, tag=f"r_{s}", name=f"r_{s}")
            nc.vector.tensor_reduce(
                out=r, in_=ps.rearrange("p (b c) -> p b c", c=C2),
                axis=mybir.AxisListType.X, op=mybir.AluOpType.add,
            )
            tmp = pool.tile([NF, BS], F32, tag=f"t_{s}", name=f"t_{s}")
            nc.vector.tensor_scalar_mul(tmp, r, SLOPE)
            nc.vector.tensor_max(out=out_sb[:, s * BS:(s + 1) * BS],
                                 in0=r, in1=tmp)
            nc.scalar.dma_start(
                out=out[:, s * BS:(s + 1) * BS],
                in_=out_sb[:, s * BS:(s + 1) * BS],
            )


def _build():
    global _cached
    if _cached is not None:
        return _cached
    nc = bacc.Bacc(
        "TRN2",
        target_bir_lowering=False,
        debug=False,
        enable_asserts=False,
        num_devices=NCORES,
    )
    x_ap = nc.dram_tensor(
        "x", [NTILE - 1, P, TILE_W], F32R, kind="ExternalInput"
    ).ap()
    xt_ap = nc.dram_tensor("xtail", [QUAD, P, CW], F32R, kind="ExternalInput").ap()
    w_ap = nc.dram_tensor("w", [P, NSW * NCHUNK * NF], F32R, kind="ExternalInput").ap()
    b_ap = nc.dram_tensor("bias4", [1, OUT_F], F32R, kind="ExternalInput").ap()
    ones_ap = nc.dram_tensor("ones", [1, HALF], F32R, kind="ExternalInput").ap()
    out_ap = nc.dram_tensor("out", [NF, NSW * BS], F32, kind="ExternalOutput").ap()
    with tile.TileContext(nc, trace_sim=False) as tc:
        _kernel_body(tc, x_ap, xt_ap, w_ap, b_ap, ones_ap, out_ap)
    nc.compile()
    _cached = nc
    return nc


def kernel(x, W, b):
    global last_results
    assert x.shape == (B, 1, NSW * NBINS * NDCT, HW, HW), x.shape
    nc = _build()

    # Host-side folding of the DCT matrices into the conv weights (tiny).
    Ct = _dct2(NDCT)                       # [f, t]
    Cs = _dct2(HW)                         # [p, i]
    Weff = np.einsum(
        "ft,pi,qj,sofpq->sotij", Ct, Cs, Cs, W.astype(np.float64), optimize=True
    ) / float(NBINS)
    Weff_k = Weff.reshape(NSW, NF, K)      # [s, o, k]
    # device layout: w[p, (s*NCHUNK + ch)*NF + o] = Weff_k[s, o, ch*128 + p]
    w_dev = np.ascontiguousarray(
        Weff_k.reshape(NSW, NF, NCHUNK, P).transpose(3, 0, 2, 1).reshape(P, NSW * NCHUNK * NF)
    ).astype(np.float32)
    bias4_dev = np.ascontiguousarray(
        (b.astype(np.float64) / C2).reshape(1, OUT_F)
    ).astype(np.float32)

    # x[b, 0, t_global, i, j]; t_global = s*256 + c*32 + t; k = t*64 + ij,
    # chunk ch = k // 128, kin = k % 128 = (t % 2)*64 + ij; c = cc*4 + c2
    x2 = x.reshape(B, NSW, 2, C2, NCHUNK, 2, HW * HW)  # (b, s, cc, c2, ch, th, ij)
    in_maps = []
    for i in range(NCORES):
        xs = x2[i * BS:(i + 1) * BS]
        # -> [s, ch, kin=(th,ij), cc, b, c2], then quad-group the chunks
        flat = np.ascontiguousarray(
            xs.transpose(1, 4, 5, 6, 2, 0, 3)   # [s, ch, th, ij, cc, b, c2]
            .reshape(NSW, NQ, QUAD, P, CW)
            .transpose(0, 1, 3, 2, 4)           # [s, q, kin, ch4, (cc b c2)]
        ).reshape(NTILE, P, TILE_W)
        xtail = np.ascontiguousarray(
            flat[NTILE - 1].reshape(P, QUAD, CW).transpose(1, 0, 2)
        )
        in_maps.append({"x": flat[:NTILE - 1], "xtail": xtail,
                        "w": w_dev, "bias4": bias4_dev,
                        "ones": np.ones((1, HALF), dtype=np.float32)})
    res = run_bass_kernel_spmd(nc, in_maps, core_ids=list(range(NCORES)))
    last_results = res
    # out[o, s*BS + b] -> feat[b, s*NF + o]
    return np.ascontiguousarray(
        np.concatenate(
            [r["out"].reshape(NF, NSW, BS).transpose(2, 1, 0) for r in res.results],
            axis=0,
        ).reshape(B, OUT_F)
    )
